# revision 8
# baseline (speedup 1.0000x reference)
"""CoLaLoLa (gnn_message_passing) Trainium2 Bass kernel.

Strategy
--------
Pure data parallel over 8 NeuronCores: batch B=2048 -> 256 rows/core.

Math restructure (avoids the [B,128,128,4] pairwise tensor entirely):
  distances[b,n,m] = masses[b,n] + masses[b,m] - 2*sum_i M_i cv[b,n,i] cv[b,m,i]
  => weighted_d[b,n] = masses[b,n]*rowsum_w[n] + (w_dist @ masses[b])[n]
                       - sum_i M_i cv[b,n,i] * (2*w_dist @ cv[b,:,i])[n]

Everything is computed feature-major ([feature_partition, batch_free]) so all
contractions are TensorE matmuls with host-prefused stationary weights:
  A_cv = combo.T                  (cv_i = combo @ v_i)
  A_u  = (2*w_dist @ combo).T     (u_i  = 2*w_dist @ cv_i)
  A_e  = (w_ener @ combo).T       (weighted_e)
  A_p  = (w_pid  @ combo).T       (weighted_pz)
BatchNorm needs global batch stats -> two launches with a tiny host reduction
in between; the BN scale/shift is folded into W1 on the host between launches.
"""
import sys

sys.path.insert(0, "/opt/trn_rl_repo")

from contextlib import ExitStack

import numpy as np

import concourse.bass as bass
import concourse.mybir as mybir
import concourse.tile as tile
from concourse.bass_utils import run_bass_kernel_spmd
from concourse.vector_clock import ScopedClock

F32 = mybir.dt.float32
F32R = mybir.dt.float32r
ALU = mybir.AluOpType
ACTF = mybir.ActivationFunctionType

B, NOBJ, NCOMBO, NTOT, HID, NOUT = 2048, 50, 78, 128, 200, 2
NCORES = 8
BC = B // NCORES  # 256 batch rows per core
EPS = 1e-5
H2 = HID - 128

# matmul operand dtype: float32 (exact, 4cy/row) or float32r (~1cy/row, relaxed)
MM_DT = F32


def _patch_tail_drain():
    """walrus in this container accepts only ONE sync-wait per Drain; Tile's
    tail drain aggregates one wait per active processor.  Split it into a
    chain of single-wait drains."""
    if getattr(tile.TileContext, "_drain_patched", False):
        return

    def _drain_and_barrier(self, tick_clock, wait_clock):
        nc = self.nc
        drain_inst = nc.sync.drain()
        wait_clock.add_sem_waits(
            drain_inst.ins, ScopedClock({None: tick_clock.global_clock})
        )
        si = drain_inst.ins.sync_info
        waits = list(si.on_wait) if si is not None else []
        if len(waits) > 1:
            si.on_wait = waits[:1]
            for w in waits[1:]:
                d2 = nc.sync.drain()
                d2.ins.sync_info = mybir.SyncInfo(on_wait=[w], on_update=[])
        nc.all_engine_barrier()
        assert self.sems is not None
        popped = nc._tile_sem_poison_stack.pop()
        assert popped is self._sem_poison
        nc.clear_and_free_semaphores(list(self.sems.allocated().values()))
        nc.all_engine_barrier()

    tile.TileContext._drain_and_barrier = _drain_and_barrier
    tile.TileContext._drain_patched = True


_WSPLIT_N = [0]


def _split_multi_waits(nc):
    """walrus here accepts only ONE sync-wait per instruction; Tile can emit
    several.  Hoist extras onto same-engine NoOps inserted just before."""
    for fn in nc.m.functions:
        for bb in fn.blocks:
            out = []
            changed = False
            for inst in bb.instructions:
                si = inst.sync_info
                waits = list(si.on_wait) if si is not None else []
                if len(waits) > 1:
                    changed = True
                    for w in waits[:-1]:
                        _WSPLIT_N[0] += 1
                        nop = mybir.InstEventSemaphore(
                            name=f"wsplit-{_WSPLIT_N[0]}", ins=[], outs=[]
                        )
                        nop.engine = inst.engine
                        nop.sync_info = mybir.SyncInfo(on_wait=[w], on_update=[])
                        out.append(nop)
                    si.on_wait = waits[-1:]
                out.append(inst)
            if changed:
                bb.instructions = out


def build_launch1(iters: int = 1):
    """Per core: vec [BC,200] -> feats [5,128,BC] (comp-major) + stats [128,10]
    (cols 0..4 batch-sums of each feat component, 5..9 sums of squares)."""
    _patch_tail_drain()
    nc = bass.Bass(trn_type="TRN2")

    vec_d = nc.dram_tensor("vec", [BC, 4 * NOBJ], MM_DT, kind="ExternalInput")
    ident_d = nc.dram_tensor("ident", [128, 128], MM_DT, kind="ExternalInput")
    acv_d = nc.dram_tensor("A_cv", [NOBJ, NTOT], MM_DT, kind="ExternalInput")
    au_d = nc.dram_tensor("A_u", [NOBJ, NTOT], MM_DT, kind="ExternalInput")
    ae_d = nc.dram_tensor("A_e", [NOBJ, NTOT], MM_DT, kind="ExternalInput")
    ap_d = nc.dram_tensor("A_p", [NOBJ, NTOT], MM_DT, kind="ExternalInput")
    wdt_d = nc.dram_tensor("WdT", [NTOT, NTOT], MM_DT, kind="ExternalInput")
    rw_d = nc.dram_tensor("rw", [NTOT, 1], F32, kind="ExternalInput")
    feats_d = nc.dram_tensor("feats", [5, NTOT, BC], F32, kind="ExternalOutput")
    stats_d = nc.dram_tensor("stats", [NTOT, 10], F32, kind="ExternalOutput")

    with tile.TileContext(nc) as tc, ExitStack() as ctx:
        consts = ctx.enter_context(tc.tile_pool(name="consts", bufs=1))
        vpool = ctx.enter_context(tc.tile_pool(name="vpool", bufs=3))
        vtpool = ctx.enter_context(tc.tile_pool(name="vtpool", bufs=2))
        work = ctx.enter_context(tc.tile_pool(name="work", bufs=2))
        feats_pool = ctx.enter_context(tc.tile_pool(name="featsp", bufs=2))
        stats_pool = ctx.enter_context(tc.tile_pool(name="statsp", bufs=2))
        pt_ps = ctx.enter_context(tc.tile_pool(name="pt", bufs=2, space="PSUM"))
        mm_ps = ctx.enter_context(tc.tile_pool(name="mm", bufs=5, space="PSUM"))

        ident = consts.tile([128, 128], MM_DT, tag="ident")
        nc.sync.dma_start(ident[:], ident_d[:])
        acv = consts.tile([NOBJ, NTOT], MM_DT, tag="acv")
        nc.sync.dma_start(acv[:], acv_d[:])
        au = consts.tile([NOBJ, NTOT], MM_DT, tag="au")
        nc.sync.dma_start(au[:], au_d[:])
        ae = consts.tile([NOBJ, NTOT], MM_DT, tag="ae")
        nc.sync.dma_start(ae[:], ae_d[:])
        apw = consts.tile([NOBJ, NTOT], MM_DT, tag="apw")
        nc.sync.dma_start(apw[:], ap_d[:])
        wdt = consts.tile([NTOT, NTOT], MM_DT, tag="wdt")
        nc.sync.dma_start(wdt[:], wdt_d[:])
        rw = consts.tile([NTOT, 1], F32, tag="rw")
        nc.sync.dma_start(rw[:], rw_d[:])

        nblk = BC // 128

        for _ in range(iters):
            # ---- load + transpose input into component tiles vT_i [50, BC]
            vt = [
                vtpool.tile([NOBJ, BC], MM_DT, tag=f"vt{i}", name=f"vt{i}")
                for i in range(4)
            ]
            for blk in range(nblk):
                vload = vpool.tile([128, 4 * NOBJ], MM_DT, tag="vload")
                nc.sync.dma_start(vload[:], vec_d[blk * 128 : (blk + 1) * 128, :])
                v3 = vload.rearrange("p (j c) -> p c j", c=4)
                for i in range(4):
                    pt = pt_ps.tile([NOBJ, 128], F32, tag="pt")
                    nc.tensor.transpose(pt[:], v3[:, i, :], ident[:])
                    nc.scalar.copy(vt[i][:, blk * 128 : (blk + 1) * 128], pt[:])

            stats = stats_pool.tile([NTOT, 16], F32, tag="stats")

            # ---- matmuls + elementwise
            sq = []  # cv_i^2 tiles
            cm = []  # cv_i * u_i tiles
            for i in range(4):
                cvp = mm_ps.tile([NTOT, BC], F32, tag="mm")
                nc.tensor.matmul(cvp[:], acv[:], vt[i][:], start=True, stop=True)
                up = mm_ps.tile([NTOT, BC], F32, tag="mm")
                nc.tensor.matmul(up[:], au[:], vt[i][:], start=True, stop=True)
                s = work.tile([NTOT, BC], F32, tag=f"sq{i}")
                nc.scalar.square(s[:], cvp[:])
                cv_sb = work.tile([NTOT, BC], F32, tag=f"cvs{i}")
                nc.scalar.copy(cv_sb[:], cvp[:])
                c = work.tile([NTOT, BC], F32, tag=f"cm{i}")
                nc.vector.tensor_tensor(c[:], cv_sb[:], up[:], op=ALU.mult)
                sq.append(s)
                cm.append(c)

            f_e = feats_pool.tile([NTOT, BC], F32, tag="f_e")
            ep = mm_ps.tile([NTOT, BC], F32, tag="mm")
            nc.tensor.matmul(ep[:], ae[:], vt[0][:], start=True, stop=True)
            nc.scalar.activation(f_e[:], ep[:], ACTF.Copy, accum_out=stats[:, 2:3])
            f_pz = feats_pool.tile([NTOT, BC], F32, tag="f_pz")
            pzp = mm_ps.tile([NTOT, BC], F32, tag="mm")
            nc.tensor.matmul(pzp[:], apw[:], vt[3][:], start=True, stop=True)
            nc.scalar.activation(f_pz[:], pzp[:], ACTF.Copy, accum_out=stats[:, 4:5])

            # ptsq = sq1 + sq2 ; masses = (sq3 - sq0) - ptsq
            f_ptsq = feats_pool.tile([NTOT, BC], F32, tag="f_ptsq")
            nc.vector.scalar_tensor_tensor(
                out=f_ptsq[:], in0=sq[1][:], scalar=1.0, in1=sq[2][:],
                op0=ALU.mult, op1=ALU.add, accum_out=stats[:, 1:2],
            )
            m1 = work.tile([NTOT, BC], F32, tag="m1")
            nc.vector.tensor_tensor(m1[:], sq[3][:], sq[0][:], op=ALU.subtract)
            f_m = feats_pool.tile([NTOT, BC], F32, tag="f_m")
            nc.vector.scalar_tensor_tensor(
                out=f_m[:], in0=m1[:], scalar=1.0, in1=f_ptsq[:],
                op0=ALU.mult, op1=ALU.subtract, accum_out=stats[:, 0:1],
            )

            # wd = masses*rw + w_dist@masses - (cm3 - cm0 - cm1 - cm2)
            wd2p = mm_ps.tile([NTOT, BC], F32, tag="mm")
            if MM_DT is not F32:
                mass_mm = feats_pool.tile([NTOT, BC], MM_DT, tag="f_m_mm")
                nc.vector.tensor_copy(mass_mm[:], f_m[:])
                mm_src = mass_mm
            else:
                mm_src = f_m
            nc.tensor.matmul(wd2p[:], wdt[:], mm_src[:], start=True, stop=True)

            x1 = work.tile([NTOT, BC], F32, tag="x1")
            nc.vector.tensor_tensor(x1[:], cm[3][:], cm[0][:], op=ALU.subtract)
            x2 = work.tile([NTOT, BC], F32, tag="x2")
            nc.vector.tensor_tensor(x2[:], cm[1][:], cm[2][:], op=ALU.add)
            wd_t = work.tile([NTOT, BC], F32, tag="wd_t")
            nc.vector.scalar_tensor_tensor(
                out=wd_t[:], in0=f_m[:], scalar=rw[:, 0:1], in1=wd2p[:],
                op0=ALU.mult, op1=ALU.add,
            )
            w1t = work.tile([NTOT, BC], F32, tag="w1t")
            nc.vector.tensor_tensor(w1t[:], wd_t[:], x1[:], op=ALU.subtract)
            f_wd = feats_pool.tile([NTOT, BC], F32, tag="f_wd")
            nc.vector.scalar_tensor_tensor(
                out=f_wd[:], in0=w1t[:], scalar=1.0, in1=x2[:],
                op0=ALU.mult, op1=ALU.add, accum_out=stats[:, 3:4],
            )

            # sums of squares via ScalarE Square + accumulate
            f_list = [f_m, f_ptsq, f_e, f_wd, f_pz]
            for k, f in enumerate(f_list):
                scr = work.tile([NTOT, BC], F32, tag="scr")
                nc.scalar.activation(
                    scr[:], f[:], ACTF.Square, accum_out=stats[:, 5 + k : 6 + k]
                )

            for k, f in enumerate(f_list):
                nc.sync.dma_start(feats_d[k], f[:])
            nc.sync.dma_start(stats_d[:], stats[:, 0:10])

    _split_multi_waits(nc)
    return nc


def build_launch2(iters: int = 1):
    """Per core: featsn [5,128,BC] (+ BN folded into W1s on host) -> y [BC,2]."""
    _patch_tail_drain()
    nc = bass.Bass(trn_type="TRN2")

    feats_d = nc.dram_tensor("featsn", [5, NTOT, BC], MM_DT, kind="ExternalInput")
    w1a_d = nc.dram_tensor("W1a", [128, 5, 128], MM_DT, kind="ExternalInput")
    w1b_d = nc.dram_tensor("W1b", [128, 5, H2], MM_DT, kind="ExternalInput")
    c1a_d = nc.dram_tensor("c1a", [128, 1], F32, kind="ExternalInput")
    c1b_d = nc.dram_tensor("c1b", [H2, 1], F32, kind="ExternalInput")
    w2a_d = nc.dram_tensor("W2a", [128, NOUT], MM_DT, kind="ExternalInput")
    w2b_d = nc.dram_tensor("W2b", [H2, NOUT], MM_DT, kind="ExternalInput")
    b2_d = nc.dram_tensor("b2", [NOUT, 1], F32, kind="ExternalInput")
    id2_d = nc.dram_tensor("id2", [NOUT, NOUT], MM_DT, kind="ExternalInput")
    y_d = nc.dram_tensor("y", [BC, NOUT], F32, kind="ExternalOutput")

    with tile.TileContext(nc) as tc, ExitStack() as ctx:
        consts = ctx.enter_context(tc.tile_pool(name="consts", bufs=1))
        fpool = ctx.enter_context(tc.tile_pool(name="fpool", bufs=2))
        work = ctx.enter_context(tc.tile_pool(name="work", bufs=2))
        opool = ctx.enter_context(tc.tile_pool(name="opool", bufs=2))
        h_ps = ctx.enter_context(tc.tile_pool(name="hps", bufs=2, space="PSUM"))
        o_ps = ctx.enter_context(tc.tile_pool(name="ops", bufs=2, space="PSUM"))
        t_ps = ctx.enter_context(tc.tile_pool(name="tps", bufs=2, space="PSUM"))

        w1a = consts.tile([128, 5, 128], MM_DT, tag="w1a")
        nc.sync.dma_start(w1a[:], w1a_d[:])
        w1b = consts.tile([128, 5, H2], MM_DT, tag="w1b")
        nc.sync.dma_start(w1b[:], w1b_d[:])
        c1a = consts.tile([128, 1], F32, tag="c1a")
        nc.sync.dma_start(c1a[:], c1a_d[:])
        c1b = consts.tile([H2, 1], F32, tag="c1b")
        nc.sync.dma_start(c1b[:], c1b_d[:])
        w2a = consts.tile([128, NOUT], MM_DT, tag="w2a")
        nc.sync.dma_start(w2a[:], w2a_d[:])
        w2b = consts.tile([H2, NOUT], MM_DT, tag="w2b")
        nc.sync.dma_start(w2b[:], w2b_d[:])
        b2t = consts.tile([NOUT, 1], F32, tag="b2t")
        nc.sync.dma_start(b2t[:], b2_d[:])
        id2 = consts.tile([NOUT, NOUT], MM_DT, tag="id2")
        nc.sync.dma_start(id2[:], id2_d[:])

        for _ in range(iters):
            nf = []
            for k in range(5):
                f = fpool.tile([NTOT, BC], MM_DT, tag=f"nf{k}")
                nc.sync.dma_start(f[:], feats_d[k])
                nf.append(f)

            ph1 = h_ps.tile([128, BC], F32, tag="ph1")
            for k in range(5):
                nc.tensor.matmul(
                    ph1[:], w1a[:, k, :], nf[k][:], start=(k == 0), stop=(k == 4)
                )
            ph2 = h_ps.tile([H2, BC], F32, tag="ph2")
            for k in range(5):
                nc.tensor.matmul(
                    ph2[:], w1b[:, k, :], nf[k][:], start=(k == 0), stop=(k == 4)
                )

            hA = work.tile([128, BC], MM_DT, tag="hA")
            nc.scalar.activation(hA[:], ph1[:], ACTF.Relu, bias=c1a[:, 0:1])
            hB = work.tile([H2, BC], MM_DT, tag="hB")
            nc.scalar.activation(hB[:], ph2[:], ACTF.Relu, bias=c1b[:, 0:1])

            po = o_ps.tile([NOUT, BC], F32, tag="po")
            nc.tensor.matmul(po[:], w2a[:], hA[:], start=True, stop=False)
            nc.tensor.matmul(po[:], w2b[:], hB[:], start=False, stop=True)

            so = work.tile([NOUT, BC], MM_DT, tag="so")
            nc.scalar.activation(so[:], po[:], ACTF.Sigmoid, bias=b2t[:, 0:1])

            for blk in range(BC // 128):
                pto = t_ps.tile([128, NOUT], F32, tag="pto")
                nc.tensor.transpose(
                    pto[:], so[:, blk * 128 : (blk + 1) * 128], id2[:]
                )
                ob = opool.tile([128, NOUT], F32, tag="ob")
                nc.scalar.copy(ob[:], pto[:])
                nc.sync.dma_start(y_d[blk * 128 : (blk + 1) * 128, :], ob[:])

    _split_multi_waits(nc)
    return nc


def _host_prep1(w_combo, w_dist, w_ener, w_pid):
    combo = np.concatenate(
        [np.eye(NOBJ, dtype=np.float32), w_combo.astype(np.float32)], axis=0
    )  # [128, 50]
    return dict(
        ident=np.eye(128, dtype=np.float32),
        A_cv=np.ascontiguousarray(combo.T),
        A_u=np.ascontiguousarray((2.0 * (w_dist @ combo)).T.astype(np.float32)),
        A_e=np.ascontiguousarray((w_ener @ combo).T.astype(np.float32)),
        A_p=np.ascontiguousarray((w_pid @ combo).T.astype(np.float32)),
        WdT=np.ascontiguousarray(w_dist.T.astype(np.float32)),
        rw=np.ascontiguousarray(
            w_dist.sum(axis=1, dtype=np.float32).reshape(NTOT, 1)
        ),
    )


# comp-major index f' = k*128 + n  ->  original feature f = 5n + k
_PERM = np.array(
    [5 * (f % NTOT) + (f // NTOT) for f in range(5 * NTOT)], dtype=np.int64
)


def _host_prep2(stats_list, gamma, beta, W1, b1, W2, b2):
    S = np.sum(np.stack(stats_list, 0), axis=0)  # [128, 10]
    S1 = np.ascontiguousarray(S[:, 0:5].T).reshape(5 * NTOT)  # comp-major sums
    S2 = np.ascontiguousarray(S[:, 5:10].T).reshape(5 * NTOT)
    meanp = S1 / B
    varp = S2 / B - meanp * meanp
    gp = gamma[_PERM].astype(np.float32)
    bp = beta[_PERM].astype(np.float32)
    W1p = W1[_PERM, :].astype(np.float32)  # [640, 200]
    a = (gp / np.sqrt(varp + EPS)).astype(np.float32)
    d = (bp - meanp * a).astype(np.float32)
    W1s = (a[:, None] * W1p).astype(np.float32)
    c1 = (W1p.T @ d + b1).astype(np.float32)  # [200]
    W1s3 = W1s.reshape(5, NTOT, HID).transpose(1, 0, 2)  # [128, 5, 200]
    return dict(
        W1a=np.ascontiguousarray(W1s3[:, :, 0:128]),
        W1b=np.ascontiguousarray(W1s3[:, :, 128:HID]),
        c1a=np.ascontiguousarray(c1[0:128].reshape(128, 1)),
        c1b=np.ascontiguousarray(c1[128:HID].reshape(H2, 1)),
        W2a=np.ascontiguousarray(W2[0:128, :].astype(np.float32)),
        W2b=np.ascontiguousarray(W2[128:HID, :].astype(np.float32)),
        b2=np.ascontiguousarray(b2.reshape(NOUT, 1).astype(np.float32)),
        id2=np.eye(NOUT, dtype=np.float32),
    )


_CACHE = {}


def _get_kernels(iters: int = 1):
    key = ("k", iters)
    if key not in _CACHE:
        _CACHE[key] = (build_launch1(iters), build_launch2(iters))
    return _CACHE[key]


def kernel(vectors, w_combo, w_dist, w_ener, w_pid, gamma, beta, W1, b1, W2, b2):
    vectors = np.asarray(vectors, dtype=np.float32)
    nc1, nc2 = _get_kernels()
    consts1 = _host_prep1(
        np.asarray(w_combo, np.float32),
        np.asarray(w_dist, np.float32),
        np.asarray(w_ener, np.float32),
        np.asarray(w_pid, np.float32),
    )
    in_maps1 = [
        {"vec": np.ascontiguousarray(vectors[c * BC : (c + 1) * BC]), **consts1}
        for c in range(NCORES)
    ]
    r1 = run_bass_kernel_spmd(nc1, in_maps1, core_ids=list(range(NCORES)))
    stats_list = [r1.results[c]["stats"] for c in range(NCORES)]
    consts2 = _host_prep2(
        stats_list,
        np.asarray(gamma, np.float32),
        np.asarray(beta, np.float32),
        np.asarray(W1, np.float32),
        np.asarray(b1, np.float32),
        np.asarray(W2, np.float32),
        np.asarray(b2, np.float32),
    )
    in_maps2 = [
        {"featsn": r1.results[c]["feats"], **consts2} for c in range(NCORES)
    ]
    r2 = run_bass_kernel_spmd(nc2, in_maps2, core_ids=list(range(NCORES)))
    return np.concatenate([r2.results[c]["y"] for c in range(NCORES)], axis=0)


if __name__ == "__main__":
    np.random.seed(0)
    inputs = {
        "vectors": np.random.randn(B, 4 * NOBJ).astype(np.float32),
        "w_combo": np.random.randn(NCOMBO, NOBJ).astype(np.float32),
        "w_dist": np.random.randn(NTOT, NTOT).astype(np.float32),
        "w_ener": np.random.randn(NTOT, NTOT).astype(np.float32),
        "w_pid": np.random.randn(NTOT, NTOT).astype(np.float32),
        "gamma": np.ones(5 * NTOT, np.float32),
        "beta": np.zeros(5 * NTOT, np.float32),
        "W1": np.random.randn(5 * NTOT, HID).astype(np.float32) / 25.3,
        "b1": np.zeros(HID, np.float32),
        "W2": np.random.randn(HID, NOUT).astype(np.float32) / 14.1,
        "b2": np.zeros(NOUT, np.float32),
    }
    out = kernel(**inputs)
    print("out", out.shape, out.dtype, out[:2])
